# revision 1
# baseline (speedup 1.0000x reference)
"""3x3 median filter (reflect padding) on Trainium2, 8-core data parallel.

Input  x: (4, 3, 1024, 1024) float32
Output  : (4, 3, 1024, 1024) float32  (Kornia MedianBlur semantics)

Strategy:
  - Host: reflect-pad H and W by 1 -> (12, 1026, 1026); shard H across 8
    cores with 1-row halo: core k gets padded rows [128k, 128k+130).
  - Device (per core): for each of the 12 images, load three partition-
    aligned tiles T0/T1/T2 = rows p, p+1, p+2 (so vertical neighbors are
    partition-aligned, no cross-partition ops needed), then the classic
    sorted-column median network (18 min/max ops/pixel amortized):
      column sort (vertical):  lo/mid/hi of each 3-row column
      horizontal merge:        med3(max3(lo), med3(mid), min3(hi))
    All min/max in fp32 -> result is bit-exact vs the reference.
"""

import sys

sys.path.insert(0, "/opt/trn_rl_repo")

import numpy as np

B, C, H, W = 4, 3, 1024, 1024
NIMG = B * C            # 12
NCORES = 8
ROWS_PER_CORE = H // NCORES   # 128
WP = W + 2              # 1026 padded width
HP_CORE = ROWS_PER_CORE + 2   # 130 padded rows per core

_PROGRAM = None
LAST_RESULT = None


def _build_program():
    import concourse.bacc as bacc
    import concourse.tile as tile
    import concourse.mybir as mybir
    from contextlib import ExitStack

    f32 = mybir.dt.float32
    mn = mybir.AluOpType.min
    mx = mybir.AluOpType.max

    nc = bacc.Bacc("TRN2", target_bir_lowering=False, debug=False,
                   num_devices=NCORES)
    x = nc.dram_tensor("x", [NIMG, HP_CORE, WP], f32, kind="ExternalInput").ap()
    y = nc.dram_tensor("y", [NIMG, ROWS_PER_CORE, W], f32,
                       kind="ExternalOutput").ap()

    P = ROWS_PER_CORE  # 128 partitions

    with tile.TileContext(nc) as tc, ExitStack() as ctx:
        pool = ctx.enter_context(tc.tile_pool(name="p", bufs=2))

        def tt(dst, a, b, op):
            nc.any.tensor_tensor(dst, a, b, op=op)

        for i in range(NIMG):
            T0 = pool.tile([P, WP], f32, tag="T0")
            T1 = pool.tile([P, WP], f32, tag="T1")
            T2 = pool.tile([P, WP], f32, tag="T2")
            nc.sync.dma_start(T0[:], x[i, 0:P, :])
            nc.sync.dma_start(T1[:], x[i, 1:P + 1, :])
            nc.sync.dma_start(T2[:], x[i, 2:P + 2, :])

            # vertical column sort: lo/mid/hi over rows (p, p+1, p+2)
            m = pool.tile([P, WP], f32, tag="m")
            M = pool.tile([P, WP], f32, tag="M")
            tt(m[:], T0[:], T1[:], mn)
            tt(M[:], T0[:], T1[:], mx)
            lo = pool.tile([P, WP], f32, tag="lo")
            mm = pool.tile([P, WP], f32, tag="mm")
            hi = pool.tile([P, WP], f32, tag="hi")
            mid = pool.tile([P, WP], f32, tag="mid")
            tt(lo[:], m[:], T2[:], mn)
            tt(mm[:], M[:], T2[:], mn)
            tt(hi[:], M[:], T2[:], mx)
            tt(mid[:], m[:], mm[:], mx)

            # horizontal: A = max3(lo), C = min3(hi), B = med3(mid)
            pa = pool.tile([P, W + 1], f32, tag="pa")
            A = pool.tile([P, W], f32, tag="A")
            tt(pa[:], lo[:, 0:W + 1], lo[:, 1:W + 2], mx)
            tt(A[:], pa[:, 0:W], lo[:, 2:W + 2], mx)

            pc = pool.tile([P, W + 1], f32, tag="pc")
            Cm = pool.tile([P, W], f32, tag="Cm")
            tt(pc[:], hi[:, 0:W + 1], hi[:, 1:W + 2], mn)
            tt(Cm[:], pc[:, 0:W], hi[:, 2:W + 2], mn)

            pm = pool.tile([P, W + 1], f32, tag="pm")
            pM = pool.tile([P, W + 1], f32, tag="pM")
            tt(pm[:], mid[:, 0:W + 1], mid[:, 1:W + 2], mn)
            tt(pM[:], mid[:, 0:W + 1], mid[:, 1:W + 2], mx)
            t2 = pool.tile([P, W], f32, tag="t2")
            Bm = pool.tile([P, W], f32, tag="Bm")
            tt(t2[:], pM[:, 0:W], mid[:, 2:W + 2], mn)
            tt(Bm[:], pm[:, 0:W], t2[:], mx)

            # final med3(A, B, C)
            m1 = pool.tile([P, W], f32, tag="m1")
            M1 = pool.tile([P, W], f32, tag="M1")
            tt(m1[:], A[:], Bm[:], mn)
            tt(M1[:], A[:], Bm[:], mx)
            t3 = pool.tile([P, W], f32, tag="t3")
            out = pool.tile([P, W], f32, tag="out")
            tt(t3[:], M1[:], Cm[:], mn)
            tt(out[:], m1[:], t3[:], mx)

            nc.sync.dma_start(y[i], out[:])

    nc.compile()
    return nc


def _get_program():
    global _PROGRAM
    if _PROGRAM is None:
        _PROGRAM = _build_program()
    return _PROGRAM


def kernel(x):
    global LAST_RESULT
    from concourse.bass_utils import run_bass_kernel_spmd
    import os

    x = np.asarray(x, dtype=np.float32)
    xp = np.pad(x.reshape(NIMG, H, W), ((0, 0), (1, 1), (1, 1)),
                mode="reflect")
    in_maps = [
        {"x": np.ascontiguousarray(
            xp[:, ROWS_PER_CORE * k: ROWS_PER_CORE * k + HP_CORE, :])}
        for k in range(NCORES)
    ]
    nc = _get_program()
    trace = bool(int(os.environ.get("MEDIAN_TRACE", "0")))
    res = run_bass_kernel_spmd(nc, in_maps, list(range(NCORES)), trace=trace)
    LAST_RESULT = res
    out = np.concatenate([res.results[k]["y"] for k in range(NCORES)], axis=1)
    return out.reshape(B, C, H, W)



# revision 2
# speedup vs baseline: 1.8042x; 1.8042x over previous
"""3x3 median filter (reflect padding) on Trainium2, 8-core data parallel.

Input  x: (4, 3, 1024, 1024) float32
Output  : (4, 3, 1024, 1024) float32  (Kornia MedianBlur semantics)

Strategy (v1 -> v2):
  - Host: convert to fp16 (DVE tensor_tensor runs 2 elem/cycle on 16-bit
    data vs 1 on fp32; max |err| ~2^-11 << 2e-2 tolerance), reflect-pad
    H and W by 1, shard H across 8 cores with 1-row halo.
  - Device (per core): classic separable median network, 18 min/max ops
    per pixel, all on the vector engine.  Images are processed 4 at a
    time concatenated along the free dim (N=4104) so the ~150-cycle
    per-instruction overhead is amortized; the 2-column junk columns at
    image junctions are never read by the final output DMA.
  - 8 SBUF buffers reused across the network's dataflow (fits bufs=2 for
    cross-chunk DMA prefetch).
"""

import sys

sys.path.insert(0, "/opt/trn_rl_repo")

import numpy as np

B, C, H, W = 4, 3, 1024, 1024
NIMG = B * C            # 12
NCORES = 8
ROWS_PER_CORE = H // NCORES   # 128
WP = W + 2              # 1026 padded width
HP_CORE = ROWS_PER_CORE + 2   # 130 padded rows per core
G = 4                   # images per chunk
NCHUNK = NIMG // G      # 3
N = G * WP              # 4104 free-dim elements per instruction

_PROGRAM = None
LAST_RESULT = None


def _build_program():
    import concourse.bacc as bacc
    import concourse.tile as tile
    import concourse.mybir as mybir
    from contextlib import ExitStack

    f16 = mybir.dt.float16
    mn = mybir.AluOpType.min
    mx = mybir.AluOpType.max

    nc = bacc.Bacc("TRN2", target_bir_lowering=False, debug=False,
                   num_devices=NCORES)
    x = nc.dram_tensor("x", [NIMG, HP_CORE, WP], f16, kind="ExternalInput").ap()
    y = nc.dram_tensor("y", [NIMG, ROWS_PER_CORE, W], f16,
                       kind="ExternalOutput").ap()

    P = ROWS_PER_CORE  # 128 partitions

    with tile.TileContext(nc) as tc, ExitStack() as ctx:
        pool = ctx.enter_context(tc.tile_pool(name="p", bufs=2))

        def tt(dst, a, b, op):
            nc.vector.tensor_tensor(dst, a, b, op=op)

        for c in range(NCHUNK):
            i0 = G * c
            # 8 rotating buffers; lifetimes laid out so each write lands on
            # a value that is fully consumed.
            Bufs = [pool.tile([P, N], f16, tag=f"B{k}", name=f"B{k}")
                    for k in range(8)]
            T0, T1, T2 = Bufs[0], Bufs[1], Bufs[2]
            for g in range(G):
                s = slice(g * WP, (g + 1) * WP)
                nc.sync.dma_start(T0[:, s], x[i0 + g, 0:P, :])
                nc.sync.dma_start(T1[:, s], x[i0 + g, 1:P + 1, :])
                nc.sync.dma_start(T2[:, s], x[i0 + g, 2:P + 2, :])

            # vertical 3-sort of rows (p, p+1, p+2): lo / mid / hi
            m_, M_ = Bufs[3], Bufs[4]
            tt(m_[:], T0[:], T1[:], mn)
            tt(M_[:], T0[:], T1[:], mx)
            lo, mm, hi = Bufs[0], Bufs[1], Bufs[5]   # T0,T1 dead
            tt(lo[:], m_[:], T2[:], mn)
            tt(mm[:], M_[:], T2[:], mn)
            tt(hi[:], M_[:], T2[:], mx)              # T2, M_ dead
            mid = Bufs[2]
            tt(mid[:], m_[:], mm[:], mx)             # m_, mm dead

            # horizontal: A = max3(lo), Cm = min3(hi), Bm = med3(mid)
            pa, A = Bufs[3], Bufs[4]
            tt(pa[:, 0:N - 1], lo[:, 0:N - 1], lo[:, 1:N], mx)
            tt(A[:, 0:N - 2], pa[:, 0:N - 2], lo[:, 2:N], mx)   # lo, pa dead
            pc, Cm = Bufs[1], Bufs[6]
            tt(pc[:, 0:N - 1], hi[:, 0:N - 1], hi[:, 1:N], mn)
            tt(Cm[:, 0:N - 2], pc[:, 0:N - 2], hi[:, 2:N], mn)  # hi, pc dead
            pm, pM = Bufs[0], Bufs[3]
            tt(pm[:, 0:N - 1], mid[:, 0:N - 1], mid[:, 1:N], mn)
            tt(pM[:, 0:N - 1], mid[:, 0:N - 1], mid[:, 1:N], mx)
            t2, Bm = Bufs[5], Bufs[7]
            tt(t2[:, 0:N - 2], pM[:, 0:N - 2], mid[:, 2:N], mn)  # pM, mid dead
            tt(Bm[:, 0:N - 2], pm[:, 0:N - 2], t2[:, 0:N - 2], mx)  # pm,t2 dead

            # final med3(A, Bm, Cm)
            m1, M1 = Bufs[0], Bufs[1]
            tt(m1[:, 0:N - 2], A[:, 0:N - 2], Bm[:, 0:N - 2], mn)
            tt(M1[:, 0:N - 2], A[:, 0:N - 2], Bm[:, 0:N - 2], mx)
            t3, out = Bufs[2], Bufs[3]
            tt(t3[:, 0:N - 2], M1[:, 0:N - 2], Cm[:, 0:N - 2], mn)
            tt(out[:, 0:N - 2], m1[:, 0:N - 2], t3[:, 0:N - 2], mx)

            for g in range(G):
                nc.sync.dma_start(y[i0 + g],
                                  out[:, g * WP: g * WP + W])

    nc.compile()
    return nc


def _get_program():
    global _PROGRAM
    if _PROGRAM is None:
        _PROGRAM = _build_program()
    return _PROGRAM


def kernel(x):
    global LAST_RESULT
    from concourse.bass_utils import run_bass_kernel_spmd
    import os

    x16 = np.asarray(x).astype(np.float16).reshape(NIMG, H, W)
    xp = np.pad(x16, ((0, 0), (1, 1), (1, 1)), mode="reflect")
    in_maps = [
        {"x": np.ascontiguousarray(
            xp[:, ROWS_PER_CORE * k: ROWS_PER_CORE * k + HP_CORE, :])}
        for k in range(NCORES)
    ]
    nc = _get_program()
    trace = bool(int(os.environ.get("MEDIAN_TRACE", "0")))
    res = run_bass_kernel_spmd(nc, in_maps, list(range(NCORES)), trace=trace)
    LAST_RESULT = res
    out = np.concatenate([res.results[k]["y"] for k in range(NCORES)], axis=1)
    return out.reshape(B, C, H, W).astype(np.float32)


# revision 4
# speedup vs baseline: 1.8209x; 1.0093x over previous
"""3x3 median filter (reflect padding) on Trainium2, 8-core data parallel.

Input  x: (4, 3, 1024, 1024) float32
Output  : (4, 3, 1024, 1024) float32  (Kornia MedianBlur semantics)

Strategy (v1 -> v2):
  - Host: convert to fp16 (DVE tensor_tensor runs 2 elem/cycle on 16-bit
    data vs 1 on fp32; max |err| ~2^-11 << 2e-2 tolerance), reflect-pad
    H and W by 1, shard H across 8 cores with 1-row halo.
  - Device (per core): classic separable median network, 18 min/max ops
    per pixel, all on the vector engine.  Images are processed 4 at a
    time concatenated along the free dim (N=4104) so the ~150-cycle
    per-instruction overhead is amortized; the 2-column junk columns at
    image junctions are never read by the final output DMA.
  - 8 SBUF buffers reused across the network's dataflow (fits bufs=2 for
    cross-chunk DMA prefetch).
"""

import sys

sys.path.insert(0, "/opt/trn_rl_repo")

import numpy as np

B, C, H, W = 4, 3, 1024, 1024
NIMG = B * C            # 12
NCORES = 8
ROWS_PER_CORE = H // NCORES   # 128
WP = W + 2              # 1026 padded width
HP_CORE = ROWS_PER_CORE + 2   # 130 padded rows per core
G = 4                   # images per chunk
NCHUNK = NIMG // G      # 3
N = G * WP              # 4104 free-dim elements per instruction

_PROGRAM = None
LAST_RESULT = None


def _build_program():
    import concourse.bacc as bacc
    import concourse.tile as tile
    import concourse.mybir as mybir
    from contextlib import ExitStack

    f16 = mybir.dt.float16
    mn = mybir.AluOpType.min
    mx = mybir.AluOpType.max

    nc = bacc.Bacc("TRN2", target_bir_lowering=False, debug=False,
                   num_devices=NCORES)
    x = nc.dram_tensor("x", [NIMG, HP_CORE, WP], f16, kind="ExternalInput").ap()
    y = nc.dram_tensor("y", [NIMG, ROWS_PER_CORE, W], f16,
                       kind="ExternalOutput").ap()

    P = ROWS_PER_CORE  # 128 partitions

    with tile.TileContext(nc) as tc, ExitStack() as ctx:
        pool = ctx.enter_context(tc.tile_pool(name="p", bufs=2))

        def tt(dst, a, b, op):
            nc.vector.tensor_tensor(dst, a, b, op=op)

        for c in range(NCHUNK):
            i0 = G * c
            # 8 rotating buffers; lifetimes laid out so each write lands on
            # a value that is fully consumed.
            Bufs = [pool.tile([P, N], f16, tag=f"B{k}", name=f"B{k}")
                    for k in range(8)]
            T0, T1, T2 = Bufs[0], Bufs[1], Bufs[2]
            # Loads issued from the otherwise-idle gpsimd/scalar sequencers
            # (sync alone paces at ~0.57us per dma_start, delaying arrival).
            for g in range(G):
                s = slice(g * WP, (g + 1) * WP)
                nc.gpsimd.dma_start(T0[:, s], x[i0 + g, 0:P, :])
                nc.scalar.dma_start(T1[:, s], x[i0 + g, 1:P + 1, :])
                nc.sync.dma_start(T2[:, s], x[i0 + g, 2:P + 2, :])

            # vertical 3-sort of rows (p, p+1, p+2): lo / mid / hi
            m_, M_ = Bufs[3], Bufs[4]
            lo, mm, hi = Bufs[0], Bufs[1], Bufs[5]   # overwrite T0,T1
            mid = Bufs[2]                            # overwrite T2
            if c == 0:
                # first chunk: per-image vertical ops so compute starts as
                # soon as image 0's two tiles land, instead of after 2.1 MB
                for g in range(G):
                    s = slice(g * WP, (g + 1) * WP)
                    tt(m_[:, s], T0[:, s], T1[:, s], mn)
                    tt(M_[:, s], T0[:, s], T1[:, s], mx)
                    tt(lo[:, s], m_[:, s], T2[:, s], mn)
                    tt(mm[:, s], M_[:, s], T2[:, s], mn)
                    tt(hi[:, s], M_[:, s], T2[:, s], mx)
                    tt(mid[:, s], m_[:, s], mm[:, s], mx)
            else:
                tt(m_[:], T0[:], T1[:], mn)
                tt(M_[:], T0[:], T1[:], mx)
                tt(lo[:], m_[:], T2[:], mn)
                tt(mm[:], M_[:], T2[:], mn)
                tt(hi[:], M_[:], T2[:], mx)          # T2, M_ dead
                tt(mid[:], m_[:], mm[:], mx)         # m_, mm dead

            # horizontal: A = max3(lo), Cm = min3(hi), Bm = med3(mid)
            pa, A = Bufs[3], Bufs[4]
            tt(pa[:, 0:N - 1], lo[:, 0:N - 1], lo[:, 1:N], mx)
            tt(A[:, 0:N - 2], pa[:, 0:N - 2], lo[:, 2:N], mx)   # lo, pa dead
            pc, Cm = Bufs[1], Bufs[6]
            tt(pc[:, 0:N - 1], hi[:, 0:N - 1], hi[:, 1:N], mn)
            tt(Cm[:, 0:N - 2], pc[:, 0:N - 2], hi[:, 2:N], mn)  # hi, pc dead
            pm, pM = Bufs[0], Bufs[3]
            tt(pm[:, 0:N - 1], mid[:, 0:N - 1], mid[:, 1:N], mn)
            tt(pM[:, 0:N - 1], mid[:, 0:N - 1], mid[:, 1:N], mx)
            t2, Bm = Bufs[5], Bufs[7]
            tt(t2[:, 0:N - 2], pM[:, 0:N - 2], mid[:, 2:N], mn)  # pM, mid dead
            tt(Bm[:, 0:N - 2], pm[:, 0:N - 2], t2[:, 0:N - 2], mx)  # pm,t2 dead

            # final med3(A, Bm, Cm)
            m1, M1 = Bufs[0], Bufs[1]
            t3, out = Bufs[2], Bufs[3]
            if c == NCHUNK - 1:
                # last chunk: per-image final stage so each image's store
                # overlaps the next image's compute instead of all stores
                # landing after the last op
                for g in range(G):
                    s = slice(g * WP, g * WP + W)
                    tt(m1[:, s], A[:, s], Bm[:, s], mn)
                    tt(M1[:, s], A[:, s], Bm[:, s], mx)
                    tt(t3[:, s], M1[:, s], Cm[:, s], mn)
                    tt(out[:, s], m1[:, s], t3[:, s], mx)
                    eng = (nc.gpsimd, nc.scalar, nc.sync, nc.gpsimd)[g]
                    eng.dma_start(y[i0 + g], out[:, s])
            else:
                tt(m1[:, 0:N - 2], A[:, 0:N - 2], Bm[:, 0:N - 2], mn)
                tt(M1[:, 0:N - 2], A[:, 0:N - 2], Bm[:, 0:N - 2], mx)
                tt(t3[:, 0:N - 2], M1[:, 0:N - 2], Cm[:, 0:N - 2], mn)
                tt(out[:, 0:N - 2], m1[:, 0:N - 2], t3[:, 0:N - 2], mx)
                for g in range(G):
                    nc.sync.dma_start(y[i0 + g],
                                      out[:, g * WP: g * WP + W])

    nc.compile()
    return nc


def _get_program():
    global _PROGRAM
    if _PROGRAM is None:
        _PROGRAM = _build_program()
    return _PROGRAM


def kernel(x):
    global LAST_RESULT
    from concourse.bass_utils import run_bass_kernel_spmd
    import os

    x16 = np.asarray(x).astype(np.float16).reshape(NIMG, H, W)
    xp = np.pad(x16, ((0, 0), (1, 1), (1, 1)), mode="reflect")
    in_maps = [
        {"x": np.ascontiguousarray(
            xp[:, ROWS_PER_CORE * k: ROWS_PER_CORE * k + HP_CORE, :])}
        for k in range(NCORES)
    ]
    nc = _get_program()
    trace = bool(int(os.environ.get("MEDIAN_TRACE", "0")))
    res = run_bass_kernel_spmd(nc, in_maps, list(range(NCORES)), trace=trace)
    LAST_RESULT = res
    out = np.concatenate([res.results[k]["y"] for k in range(NCORES)], axis=1)
    return out.reshape(B, C, H, W).astype(np.float32)
